# revision 41
# baseline (speedup 1.0000x reference)
"""CenterLoss kernel for 8x Trainium2 NeuronCores.

Algorithm (matches the jax reference):
  counts[c] = #samples of class c; sums[c,:] = sum of embeddings of class c
  means = sums / max(counts,1)
  norms[i] = ||e_i - means[t_i]||
  loss = sum_c (1/n_c) * sum_{i in c} norms[i]   (empty classes contribute 0)
       = sum_i w_{t_i} * norms[i],  w_c = (n_c>0)/n_c   <- no 2nd segment-sum

Device strategy (data-parallel over samples, N/8 per core):
  Pass 1: per TPC-tile chunk, one DMA + one fat f32->fp16 cast (ScalarE) into
    the SBUF-resident copy; dim-0 arrives pre-shifted +SHIFT from the host
    (counts channel).  Per 128-sample tile a one-hot (DVE is_equal vs iota)
    and a PE matmul accumulate sumsT[d, c] in PSUM; row 0 = SHIFT*n_c + s0_c.
  Counts are decoded locally (n = round(row0/SHIFT), s0 = row0 - SHIFT*n),
    the payload is packed to fp16 [sums(128) ; counts(1)] and AllReduced
    (258KB instead of 512KB f32).
  Table build: means = sums/max(n,1) transposed to [class, d] rows with the
    fp16 mean in bytes 0:256 and w2 = (n>0)/n^2 at f32 slot 64; DMA to DRAM.
  Pass 2: dma_gather pulls each sample's table row with a 3-deep software
    pipeline (4 gather bufs, 4 SWDGE queues) so desc-gen/DMA/compute overlap;
    per batch one fat fp16 subtract (8 tiles) + strided w2 extract; per tile
    ||e-m||^2 via square+accumulate alternating between DVE
    (scalar_tensor_tensor) and ScalarE (Square activation) into a
    [128, tiles] column buffer.  Tail: one DVE multiply, one ScalarE
    Sqrt+accum, one PE reduce -> scalar out.
  Host sums the 8 per-core scalars.
"""

import sys

import numpy as np

for _p in ("/opt/trn_rl_repo", "/root/.axon_site/_ro/trn_rl_repo"):
    if _p not in sys.path:
        sys.path.append(_p)

D, C = 128, 1000
NCORES = 8
SHIFT = 128.0  # fp8 e4m3: ulp(128)=16 so e0+128 rounds to exactly 128 (<240 max)
TPC = 8  # tiles per E-staging chunk
GB = 8  # tiles per dma_gather call (1024 idx; 2048 overflows the SWDGE ring)
PF = 3  # gather batches prefetched ahead of consumption
GAT_BUFS = None  # gather buffer count (default PF + 1)
SQ_MODE = "dve"  # "split" | "dve" | "act" — engine(s) for the square-accum

_cache = {}


def _build(n_loc, stage=3):
    import concourse.bacc as bacc
    import concourse.mybir as mybir
    import concourse.tile as tile
    from concourse import library_config

    f32 = mybir.dt.float32
    f16 = mybir.dt.float16
    f8 = mybir.dt.float8e4
    i16 = mybir.dt.int16
    i32 = mybir.dt.int32
    AF = mybir.ActivationFunctionType
    ALU = mybir.AluOpType
    AX = mybir.AxisListType
    PM = mybir.MatmulPerfMode

    tiles = n_loc // 128
    chunks = tiles // TPC

    nc = bacc.Bacc(
        "TRN2",
        target_bir_lowering=False,
        debug=False,
        enable_asserts=False,
        num_devices=NCORES,
        num_swdge_queues=4,
    )

    emb = nc.dram_tensor("emb", [n_loc, D], f32, kind="ExternalInput")
    tgtf = nc.dram_tensor("tgtf", [128, tiles], f32, kind="ExternalInput")
    gidx = nc.dram_tensor("gidx", [128, n_loc // 16], i16, kind="ExternalInput")
    iota = nc.dram_tensor("iota", [128, C], f16, kind="ExternalInput")
    ident = nc.dram_tensor("ident", [128, 128], f32, kind="ExternalInput")
    out = nc.dram_tensor("out", [1, 1], f32, kind="ExternalOutput")

    # one DMA drops TPC tiles into SBUF [128, TPC, 128]:
    # (p, j, d) <- emb[(chunk*TPC + j)*128 + p, d]
    emb_t = emb.ap().rearrange("(c j p) d -> c p j d", p=128, j=TPC)
    gcols = GB * 8  # gather-index columns per batch

    with tile.TileContext(nc) as tc:
        with (
            tc.tile_pool(name="const", bufs=1) as constp,
            tc.tile_pool(name="big", bufs=1) as bigp,
            tc.tile_pool(name="tmp1k", bufs=1) as tmp1kp,
            tc.tile_pool(name="small", bufs=2) as smallp,
            tc.tile_pool(name="acc1", bufs=1, space="PSUM") as psump,
            tc.tile_pool(name="ptr", bufs=2, space="PSUM") as psumtp,
            tc.tile_pool(name="dram", bufs=1, space="DRAM") as dramp,
        ):
            # GPSIMD library carrying the dma_gather Q7 kernel
            nc.gpsimd.load_library(library_config.mlp)

            # ---- persistent constants ----
            ident_sb = constp.tile([128, 128], f32)
            nc.sync.dma_start(ident_sb[:], ident.ap())
            ones_col = constp.tile([128, 1], f32)
            nc.vector.memset(ones_col[:], 1.0)
            ones_row = constp.tile([1, 128], f32)
            nc.vector.memset(ones_row[:], 1.0)

            resident = bigp.tile([128, tiles * D], f16, tag="resident")
            res3 = resident[:].rearrange("p (j d) -> p j d", d=D)

            psum_acc = psump.tile([128, C], f32, tag="acc")

            # =================== PASS 1 ===================
            with (
                tc.tile_pool(name="xfer", bufs=2) as xferp,
                tc.tile_pool(name="oh", bufs=2) as ohp,
                tc.tile_pool(name="w8p", bufs=2) as w8p,
                tc.tile_pool(name="p1c", bufs=1) as p1cp,
            ):
                iota_sb = p1cp.tile([128, C], f16)
                nc.sync.dma_start(iota_sb[:], iota.ap())
                tgt_sb = p1cp.tile([128, tiles], f32)
                nc.sync.dma_start(tgt_sb[:], tgtf.ap())

                for cki in range(chunks):
                    est = xferp.tile([128, TPC, D], f32, tag="xfer")
                    nc.sync.dma_start(est[:], emb_t[cki])
                    rc = resident[:, cki * TPC * D : (cki + 1) * TPC * D]
                    # fat cast f32 -> fp16 (the pass-2 resident copy); dim-0
                    # is pre-shifted by +SHIFT host-side (counts channel); in
                    # fp8 the shifted dim-0 rounds to exactly SHIFT, so PSUM
                    # row 0 = SHIFT*n exactly; the means table re-adds SHIFT
                    # so the pass-2 subtract cancels the resident shift.
                    nc.scalar.copy(rc, est[:].rearrange("p j d -> p (j d)"))
                    # fp8 weights for the DoubleRow segment-sum matmul
                    w8 = w8p.tile([128, TPC, D], f8, tag="w8")
                    nc.scalar.copy(w8[:], est[:])
                    for pr in range(TPC // 2):
                        jp = cki * TPC + pr * 2
                        # one-hot pair [128 samples, 2, 1000] fp8
                        oh8 = ohp.tile([128, 2, C], f8, tag="oh")
                        for k in range(2):
                            nc.vector.tensor_scalar(
                                oh8[:, k, :], iota_sb[:],
                                tgt_sb[:, jp + k : jp + k + 1], None,
                                op0=ALU.is_equal,
                            )
                        first = jp == 0
                        last = jp == tiles - 2
                        nc.tensor.matmul(
                            psum_acc[:, 0:512],
                            w8[:, pr * 2 : pr * 2 + 2, :],
                            oh8[:, :, 0:512],
                            start=first, stop=last, perf_mode=PM.DoubleRow,
                        )
                        nc.tensor.matmul(
                            psum_acc[:, 512:C],
                            w8[:, pr * 2 : pr * 2 + 2, :],
                            oh8[:, :, 512:C],
                            start=first, stop=last, perf_mode=PM.DoubleRow,
                        )

            if stage == 0.8:
                dbg = smallp.tile([2, 8], f32, tag="dbg")
                nc.scalar.copy(dbg[:], psum_acc[0:2, 0:8])
                o8 = nc.dram_tensor("o8", [2, 8], f32, kind="ExternalOutput")
                nc.sync.dma_start(o8.ap(), dbg[:])

            # =================== LOCAL DECODE + ALL-REDUCE (fp16) ===========
            row0 = psum_acc[0:1, :]
            n_i32 = tmp1kp.tile([1, C], i32, tag="t1k_a")
            # n = round(row0/SHIFT): frac = s0/SHIFT in (-0.25, 0.25), so
            # truncate(n + 0.25 + frac) == n under either rounding mode.
            nc.vector.tensor_scalar(
                n_i32[:], row0, 1.0 / SHIFT, 0.25, op0=ALU.mult, op1=ALU.add
            )
            n_f = constp.tile([1, C], f32)
            nc.vector.tensor_copy(n_f[:], n_i32[:])
            # s0 = row0 - SHIFT*n  (true dim-0 sums)
            neg = tmp1kp.tile([1, C], f32, tag="t1k_b")
            nc.vector.tensor_scalar_mul(neg[:], n_f[:], -SHIFT)
            nc.vector.tensor_add(neg[:], row0, neg[:])  # now holds s0

            payload = constp.tile([128, C], f16)
            nc.scalar.copy(payload[:], psum_acc[:])
            nc.vector.tensor_copy(payload[0:1, :], neg[:])
            cnt16 = tmp1kp.tile([1, C], f16, tag="t1k_d")
            nc.vector.tensor_copy(cnt16[:], n_f[:])

            ar_in = dramp.tile([129, C], f16)
            ar_out = dramp.tile([129, C], f16)
            nc.sync.dma_start(ar_in[0:128, :], payload[:])
            nc.sync.dma_start(ar_in[128:129, :], cnt16[:])
            nc.gpsimd.collective_compute(
                "AllReduce",
                ALU.add,
                replica_groups=[list(range(NCORES))],
                ins=[ar_in.opt()],
                outs=[ar_out.opt()],
            )
            gsum16 = constp.tile([128, C], f16)
            nc.sync.dma_start(gsum16[:], ar_out[0:128, :])
            gcnt16 = tmp1kp.tile([1, C], f16, tag="t1k_a")
            nc.sync.dma_start(gcnt16[:], ar_out[128:129, :])

            if stage >= 2:
                # =================== TABLE BUILD ===================
                gsums = constp.tile([128, C], f32)
                nc.vector.tensor_copy(gsums[:], gsum16[:])
                gn_f = tmp1kp.tile([1, C], f32, tag="t1k_b")
                nc.vector.tensor_copy(gn_f[:], gcnt16[:])
                # inv = 1/max(n,1); w2 = inv*inv*(n>0)
                nmax = tmp1kp.tile([1, C], f32, tag="t1k_c")
                nc.vector.tensor_scalar_max(nmax[:], gn_f[:], 1.0)
                inv = constp.tile([1, C], f32)
                nc.vector.reciprocal(inv[:], nmax[:])
                mask = tmp1kp.tile([1, C], f32, tag="t1k_c")
                nc.vector.tensor_scalar(mask[:], gn_f[:], 0.5, None, op0=ALU.is_gt)
                w2 = constp.tile([1, C], f32)
                nc.vector.tensor_mul(w2[:], inv[:], inv[:])
                nc.vector.tensor_mul(w2[:], w2[:], mask[:])

                # broadcast inv across partitions via PE outer product
                pinv = psump.tile([128, C], f32, tag="pinv")
                nc.tensor.matmul(
                    pinv[:, 0:512], ones_row[:], inv[:, 0:512],
                    start=True, stop=True,
                )
                nc.tensor.matmul(
                    pinv[:, 512:C], ones_row[:], inv[:, 512:C],
                    start=True, stop=True,
                )
                meansT = constp.tile([128, C], f32)
                nc.vector.tensor_mul(meansT[:], gsums[:], pinv[:])
                # keep dim-0 shifted so it cancels against the resident shift
                nc.vector.tensor_scalar_add(meansT[0:1, :], meansT[0:1, :], SHIFT)

                # transpose to [class, d] rows; pack fp16 means + f32 w2
                rowbuf = constp.tile([128, 8, 128], f32)
                nc.vector.memset(rowbuf[:], 0.0)
                rowbuf16 = rowbuf[:].bitcast(f16)  # [128, 8, 256]
                for c8 in range(8):
                    cl = c8 * 128
                    ncl = min(128, C - cl)
                    tp = psumtp.tile([128, 128], f32, tag="tp")
                    nc.tensor.transpose(
                        tp[0:ncl, :], meansT[:, cl : cl + ncl], ident_sb[:]
                    )
                    nc.scalar.copy(rowbuf16[0:ncl, c8, 0:128], tp[0:ncl, :])
                    tpw = psumtp.tile([128, 1], f32, tag="tpw")
                    nc.tensor.transpose(
                        tpw[0:ncl, :], w2[0:1, cl : cl + ncl],
                        ident_sb[0:1, 0:1],
                    )
                    nc.scalar.copy(rowbuf[0:ncl, c8, 64:65], tpw[0:ncl, :])

                table = nc.dram_tensor("table", [1024, 128], f32, kind="Internal")
                tbl_v = table.ap().rearrange("(c p) d -> p c d", p=128)
                nc.sync.dma_start(tbl_v, rowbuf[:])

            if stage >= 2.2:
                # =================== PASS 2 ===================
                # gathered row (fp16 view): [0:128]=mean, f32 slot 64=w2
                nsqbuf = constp.tile([128, tiles], f32)
                w2buf = constp.tile([128, tiles], f32)
                nbat = tiles // GB
                bat_per_gx = 8  # gather batches per index-chunk DMA
                gx_cols = gcols * bat_per_gx
                with (
                    tc.tile_pool(name="gat", bufs=GAT_BUFS or (PF + 1)) as gatp,
                    tc.tile_pool(name="gix", bufs=2) as gixp,
                    tc.tile_pool(name="diffp", bufs=2) as diffp,
                ):
                    gxts = {}
                    gts = {}

                    def issue(bi):
                        ci = bi // bat_per_gx
                        if bi % bat_per_gx == 0:
                            gxt = gixp.tile([128, gx_cols], i16, tag="gx")
                            nc.sync.dma_start(
                                gxt[:],
                                gidx.ap()[:, ci * gx_cols : (ci + 1) * gx_cols],
                            )
                            gxts[ci] = gxt
                        gt = gatp.tile([128, GB, 128], f32, tag="gt")
                        nc.gpsimd.dma_gather(
                            gt[:],
                            table.ap(),
                            gxts[ci][:, (bi % bat_per_gx) * gcols :
                                     (bi % bat_per_gx + 1) * gcols],
                            num_idxs=GB * 128,
                            num_idxs_reg=GB * 128,
                            elem_size=128,
                            queue_num=bi % 4,
                        )
                        gts[bi] = gt

                    for bi in range(min(PF, nbat)):
                        issue(bi)
                    for bi in range(nbat):
                        if bi + PF < nbat:
                            issue(bi + PF)
                        gt = gts.pop(bi)
                        gt16 = gt[:].bitcast(f16)  # [128, GB, 256]
                        if stage < 2.4:
                            continue
                        # fat fp16 subtract over the whole batch (8 tiles)
                        diff8 = diffp.tile([128, GB, D], f16, tag="diff8")
                        nc.vector.tensor_sub(
                            diff8[:],
                            res3[:, bi * GB : (bi + 1) * GB, :],
                            gt16[:, :, 0:128],
                        )
                        if stage < 2.5:
                            continue
                        # w2 per sample, strided across the batch
                        nc.vector.tensor_copy(
                            w2buf[:, bi * GB : (bi + 1) * GB],
                            gt[:, :, 64:65].rearrange("p j o -> p (j o)"),
                        )
                        if stage < 2.6:
                            continue
                        # ||e-m||^2 per tile: square + accumulate on DVE
                        for j16 in range(GB):
                            j = bi * GB + j16
                            sq = smallp.tile([128, D], f16, tag="sq")
                            nc.vector.scalar_tensor_tensor(
                                sq[:],
                                diff8[:, j16, :],
                                0.0,
                                diff8[:, j16, :],
                                op0=ALU.bypass,
                                op1=ALU.mult,
                                accum_out=nsqbuf[:, j : j + 1],
                            )

                if stage >= 3:
                    # =================== TAIL ===================
                    nc.vector.tensor_mul(nsqbuf[:], nsqbuf[:], w2buf[:])
                    acc = smallp.tile([128, 1], f32, tag="acc")
                    nc.scalar.activation(
                        w2buf[:], nsqbuf[:], AF.Sqrt, accum_out=acc[:]
                    )
                    fin = psumtp.tile([1, 1], f32, tag="tpw")
                    nc.tensor.matmul(
                        fin[:], acc[:], ones_col[:], start=True, stop=True
                    )
                    fin_sb = smallp.tile([1, 1], f32, tag="fin_sb")
                    nc.scalar.copy(fin_sb[:], fin[:])
                    nc.sync.dma_start(out.ap(), fin_sb[:])
                else:
                    fs = smallp.tile([1, 1], f32, tag="fin_sb")
                    nc.scalar.copy(fs[:], w2[0:1, 0:1])
                    nc.sync.dma_start(out.ap(), fs[:])
            elif stage == 2:
                fs = smallp.tile([1, 1], f32, tag="fin_sb")
                nc.scalar.copy(fs[:], w2[0:1, 0:1])
                nc.sync.dma_start(out.ap(), fs[:])
            else:
                fs = smallp.tile([1, 1], f32, tag="fin_sb")
                nc.scalar.copy(fs[:], gsum16[0:1, 0:1])
                nc.sync.dma_start(out.ap(), fs[:])

    nc.compile()
    return nc


def _host_inputs(embeddeds, target, n_loc):
    """Build the per-core input maps."""
    tiles = n_loc // 128
    iota_np = np.broadcast_to(
        np.arange(C, dtype=np.float16)[None, :], (128, C)
    ).copy()
    ident_np = np.eye(128, dtype=np.float32)
    in_maps = []
    for r in range(NCORES):
        e = np.array(embeddeds[r * n_loc : (r + 1) * n_loc])
        e[:, 0] += np.float32(SHIFT)  # counts channel: dim-0 pre-shift
        t = target[r * n_loc : (r + 1) * n_loc]
        # [128, tiles]: tgtf[p, j] = t[128j + p]
        tgtf_np = np.ascontiguousarray(t.reshape(tiles, 128).T.astype(np.float32))
        # [128, n_loc/16]: gidx[p, k] = t[16k + p%16], replicated to 128 rows
        g = t.reshape(n_loc // 16, 16).T.astype(np.int16)  # [16, n/16]
        gidx_np = np.ascontiguousarray(np.tile(g, (8, 1)))
        in_maps.append(
            {
                "emb": e,
                "tgtf": tgtf_np,
                "gidx": gidx_np,
                "iota": iota_np,
                "ident": ident_np,
            }
        )
    return in_maps


def kernel(embeddeds, target, _trace=False, _stage=3):
    from concourse import bass_utils

    embeddeds = np.asarray(embeddeds, dtype=np.float32)
    target = np.asarray(target, dtype=np.int32)
    n = embeddeds.shape[0]
    n_loc = n // NCORES

    key = (n_loc, _stage)
    if key not in _cache:
        _cache[key] = _build(n_loc, stage=_stage)
    nc = _cache[key]

    in_maps = _host_inputs(embeddeds, target, n_loc)
    res = bass_utils.run_bass_kernel_spmd(
        nc, in_maps, core_ids=list(range(NCORES)), trace=_trace
    )
    total = np.float64(0.0)
    for r in res.results:
        total += np.float64(r["out"][0, 0])
    kernel.last_results = res
    return np.asarray(np.float32(total))
